# revision 29
# baseline (speedup 1.0000x reference)
"""Ex2Vec Trainium2 Bass kernel.

Data-parallel over batch B=32 across 8 NeuronCores (4 batch rows/core).
The (augmented) item table is replicated to every core; all gathers run
on-device via indirect DMA.

Math (per batch row b):
    u   = emb_user[user]                  [D]
    p   = emb_item[pred]                  [P, D]
    h   = emb_item[hist]                  [H, D]
    sq[i,j]   = |h_i|^2 + |p_j|^2 - 2 h_i.p_j   (+EPS folded in)
    dist      = sqrt(max(sq, EPS))
    kern      = sigmoid(smooth/(1+dist) - force*smooth) / denom
    td        = (t + cutoff)^-.5 * w * (global_lamb + user_lamb[u]) / denom
    res_j     = sum_i td_i kern[i,j]
    dist_ui_j = sqrt(max(|u-p_j|^2 + EPS, EPS))
    out       = relu(dist_ui - res)
    I         = alpha*out + beta*out^2 + gamma + user_bias[u] + item_bias[pred]

Device layout notes:
  - host passes aug_item = [emb_item | item_bias]  [V, 65] and
    user_aug = [emb_user | user_lamb | user_bias]  [V, 66].
  - ONE indirect DMA per batch row gathers all 8 pred chunks AND the
    history rows ([128, 9] offsets -> [128, 9, 65]).
  - all additive distance terms (hh+EPS, pp, uu+EPS) ride the matmuls as
    augmented K-rows, so PSUM holds finished squared distances.
  - mm1 (h x p Gram) is row-space [H=128, 512]; mm2 (u-dist) and mm3
    (td-reduce) use indicator-masked lhsT columns accumulating all 4
    batch rows into one [4, 512] PSUM tile, then get transposed into
    column space [128, 32] where the final polynomial runs.
  - reciprocal = ACT Reciprocal + one Newton step (DVE mult/adds).
"""

import os
import numpy as np
from contextlib import ExitStack

import concourse.bass as bass
import concourse.bacc as bacc
import concourse.mybir as mybir
import concourse.tile as tile
from concourse.masks import make_identity
from concourse.bass_utils import run_bass_kernel_spmd

F32 = mybir.dt.float32
F32R = mybir.dt.float32r
I32 = mybir.dt.int32
AF = mybir.ActivationFunctionType
OP = mybir.AluOpType
AX = mybir.AxisListType

NCORES = 8
B = 32
BPC = B // NCORES          # 4 batch rows per core
P_REAL = 1000
PP = 1024                  # padded pred count
NCH = PP // 128            # 8 chunks of 128 pred rows
H = 128
D = 64
V = 100001
EPS = 1e-12

# tunables
F32R_MM1 = False           # f32r for the big Gram matmul
F32R_MM23 = False          # f32r for squ/res accumulate matmuls
ACT_RECIP = True           # ACT Reciprocal + 1 Newton step (else DVE recip)
PP_ON_ACT = True           # squares for |p|^2 on ACT
ADD1_ON_GPS = True         # the (d+1)/smooth pass on GPSIMD

_cache: dict = {}


def _raw_activation(nc, out, in_, func, bias=0.0, scale=1.0):
    """Emit InstActivation without the bass accuracy guard (Reciprocal)."""
    return nc.scalar.add_instruction(
        mybir.InstActivation(
            name=nc.get_next_instruction_name(),
            func=func,
            ins=[nc.scalar.lower_ap(in_),
                 mybir.ImmediateValue(dtype=F32, value=float(bias)),
                 mybir.ImmediateValue(dtype=F32, value=float(scale)),
                 mybir.ImmediateValue(dtype=F32, value=0.0)],
            outs=[nc.scalar.lower_ap(out)]))


def _build(scalars):
    """Build + return the Bass module for one core's program."""
    (global_lamb, alpha, beta, gamma, cutoff, smooth, force) = scalars
    denom = 1.0 / (1.0 + np.exp(-(smooth - force * smooth)))  # sigmoid
    inv_denom = float(1.0 / denom)
    inv_smooth = float(1.0 / smooth)
    neg_fs = float(-force * smooth)
    MDT1 = F32R if F32R_MM1 else F32
    MDT3 = F32R if F32R_MM23 else F32

    nc = bacc.Bacc("TRN2", target_bir_lowering=False, debug=False,
                   num_devices=NCORES)

    # activation-bias constants must exist as const APs before use
    for v in sorted({float(cutoff), EPS, -EPS, neg_fs}):
        if (F32, v) not in nc.const_aps.aps:
            t = nc.alloc_sbuf_tensor(f"constap-{v}", [128, 1], F32)
            nc.gpsimd.memset(t.ap(), v)
            nc.const_aps.aps[(F32, v)] = t.ap()
    nc.all_engine_barrier()

    # ---- DRAM I/O ------------------------------------------------------
    t_aug = nc.dram_tensor("aug_item", [V, 1 + D], F32, kind="ExternalInput")
    t_uaug = nc.dram_tensor("user_aug", [V, D + 2], F32, kind="ExternalInput")
    # per batch row: 8 pred chunk columns + 1 history column of offsets
    t_idxph = nc.dram_tensor("idx_ph", [128, BPC * (NCH + 1)], I32,
                             kind="ExternalInput")
    t_idxu = nc.dram_tensor("idx_user", [BPC, 1], I32, kind="ExternalInput")
    t_td = nc.dram_tensor("tdelta", [BPC, H], F32, kind="ExternalInput")
    t_wt = nc.dram_tensor("tweight", [BPC, H], F32, kind="ExternalInput")
    t_out = nc.dram_tensor("out", [BPC, PP], F32, kind="ExternalOutput")

    NCH1 = NCH + 1
    NC32 = BPC * NCH

    with tile.TileContext(nc) as tc, ExitStack() as ctx:
        const = ctx.enter_context(tc.tile_pool(name="const", bufs=1))
        sb1 = ctx.enter_context(tc.tile_pool(name="sb1", bufs=1))
        pallp = ctx.enter_context(tc.tile_pool(name="pall", bufs=4))
        psqp = ctx.enter_context(tc.tile_pool(name="psq", bufs=2))
        hsidep = ctx.enter_context(tc.tile_pool(name="hside", bufs=2))
        psidep = ctx.enter_context(tc.tile_pool(name="pside", bufs=4))
        kchain = ctx.enter_context(tc.tile_pool(name="kchain", bufs=2))
        kernp = ctx.enter_context(tc.tile_pool(name="kern", bufs=4))

        ps_pT = ctx.enter_context(tc.tile_pool(name="ps_pT", bufs=2, space="PSUM"))
        ps_mm = ctx.enter_context(tc.tile_pool(name="ps_mm", bufs=2, space="PSUM"))
        ps_acc = ctx.enter_context(tc.tile_pool(name="ps_acc", bufs=1, space="PSUM"))
        ps_cols = ctx.enter_context(tc.tile_pool(name="ps_cols", bufs=1, space="PSUM"))
        ps_misc = ctx.enter_context(tc.tile_pool(name="ps_misc", bufs=1, space="PSUM"))

        # ---- constants -------------------------------------------------
        ident = const.tile([128, 128], F32)
        make_identity(nc, ident[:])

        # ---- input loads ----------------------------------------------
        idxph = sb1.tile([128, BPC * NCH1], I32)
        idxu = sb1.tile([BPC, 1], I32)
        td_sb = sb1.tile([BPC, H], F32)
        wt_sb = sb1.tile([BPC, H], F32)
        nc.sync.dma_start(out=idxph[:], in_=t_idxph[:])
        nc.sync.dma_start(out=idxu[:], in_=t_idxu[:])
        nc.sync.dma_start(out=td_sb[:], in_=t_td[:])
        nc.sync.dma_start(out=wt_sb[:], in_=t_wt[:])

        # ---- user-side gather & prep ----------------------------------
        uaug_sb = sb1.tile([BPC, D + 2], F32)
        nc.gpsimd.indirect_dma_start(
            out=uaug_sb[:], out_offset=None, in_=t_uaug[:],
            in_offset=bass.IndirectOffsetOnAxis(ap=idxu[:, :1], axis=0))
        u_all = uaug_sb[:, 0:D]
        ul = uaug_sb[:, D:D + 1]
        ub = uaug_sb[:, D + 1:D + 2]

        # u-side: uu+eps, broadcast u rows to all partitions (outer products)
        usq = sb1.tile([BPC, D], F32)
        uu = sb1.tile([BPC, 1], F32)
        uu_eps = sb1.tile([BPC, 1], F32)
        nc.vector.tensor_mul(usq[:], u_all, u_all)
        nc.vector.reduce_sum(uu[:], usq[:], axis=AX.X)
        nc.vector.tensor_scalar_add(uu_eps[:], uu[:], EPS)
        ones_row = sb1.tile([1, 128], F32)
        nc.gpsimd.memset(ones_row[:], 1.0)
        # u_bc[:, b*64:(b+1)*64] = u_b broadcast down partitions.
        # flatten u rows onto partition 0 (DMA crosses partitions), then
        # one ones-column outer product covers all 4 batch rows.
        u_flat = sb1.tile([1, BPC * D], F32)
        nc.sync.dma_start(out=u_flat[:], in_=uaug_sb[:, 0:D])
        u_bc = sb1.tile([128, BPC * D], F32)
        ps_ubc = ps_misc.tile([128, BPC * D], F32, space="PSUM", tag="misc")
        nc.tensor.matmul(ps_ubc[:], lhsT=ones_row[:], rhs=u_flat[:],
                         start=True, stop=True)
        nc.vector.tensor_copy(u_bc[:], ps_ubc[:])
        # uueps_b[:, b] = uu_b + eps on every partition
        ps_uue = ps_misc.tile([128, BPC], F32, space="PSUM", tag="misc")
        nc.tensor.transpose(ps_uue[0:1, 0:BPC], uu_eps[:], ident[0:BPC, 0:BPC])
        uue_row = sb1.tile([1, BPC], F32)
        nc.vector.tensor_copy(uue_row[:], ps_uue[0:1, 0:BPC])
        ps_uub = ps_misc.tile([128, BPC], F32, space="PSUM", tag="misc")
        nc.tensor.matmul(ps_uub[:, 0:BPC], lhsT=ones_row[:], rhs=uue_row[:],
                         start=True, stop=True)
        uueps_b = sb1.tile([128, BPC], F32)
        nc.vector.tensor_copy(uueps_b[:], ps_uub[:, 0:BPC])

        # td row: (t+cutoff)^-0.5 * w * (gl + ulamb)/denom
        tds = sb1.tile([BPC, H], F32)
        tdr = sb1.tile([BPC, H], F32)
        tdl = sb1.tile([BPC, H], F32)
        lamb_c = sb1.tile([BPC, 1], F32)
        nc.scalar.activation(tds[:], td_sb[:], AF.Sqrt, bias=float(cutoff))
        nc.vector.reciprocal(tdr[:], tds[:])
        nc.vector.tensor_scalar(lamb_c[:], ul, float(global_lamb), inv_denom,
                                op0=OP.add, op1=OP.mult)
        nc.vector.scalar_tensor_tensor(
            out=tdl[:], in0=tdr[:], scalar=lamb_c[:, :1], in1=wt_sb[:],
            op0=OP.mult, op1=OP.mult)
        ps_td = ps_misc.tile([128, BPC], F32, space="PSUM", tag="misc")
        nc.tensor.transpose(ps_td[:, 0:BPC], tdl[:], ident[0:BPC, 0:BPC])
        td4m = sb1.tile([128, BPC * BPC], MDT3)
        nc.vector.memset(td4m[:].bitcast(F32), 0.0)
        for b in range(BPC):
            nc.vector.tensor_copy(td4m[:, b * BPC + b:b * BPC + b + 1],
                                  ps_td[:, b:b + 1])

        # gamma + user_bias broadcast [128, BPC]
        ubg = sb1.tile([BPC, 1], F32)
        nc.vector.tensor_scalar_add(ubg[:], ub, float(gamma))
        ps_ubt = ps_misc.tile([128, BPC], F32, space="PSUM", tag="misc")
        nc.tensor.transpose(ps_ubt[0:1, 0:BPC], ubg[:], ident[0:BPC, 0:BPC])
        ubg_row = sb1.tile([1, BPC], F32)
        nc.vector.tensor_copy(ubg_row[:], ps_ubt[0:1, 0:BPC])
        ps_ubb = ps_misc.tile([128, BPC], F32, space="PSUM", tag="misc")
        nc.tensor.matmul(ps_ubb[:, 0:BPC], lhsT=ones_row[:], rhs=ubg_row[:],
                         start=True, stop=True)
        ubg_b = sb1.tile([128, BPC], F32)
        nc.vector.tensor_copy(ubg_b[:], ps_ubb[:, 0:BPC])

        # ============ phase A0: gathers + |p|^2 (Square set) ===========
        p_all_tiles = []
        for b in range(BPC):
            # one gather: 8 pred chunks + history rows -> [128, 9, 65]
            p_all = pallp.tile([128, NCH1, 68], F32)
            nc.gpsimd.memset(p_all[:, 0:NCH, 1:2], 1.0)     # ones col
            for c in range(NCH1):
                nc.gpsimd.indirect_dma_start(
                    out=p_all[:, c, 2:3 + D], out_offset=None, in_=t_aug[:],
                    in_offset=bass.IndirectOffsetOnAxis(
                        ap=idxph[:, b * NCH1 + c:b * NCH1 + c + 1], axis=0))
            p_all_tiles.append(p_all)

            psq = psqp.tile([128, NCH * D], F32)
            if PP_ON_ACT:
                nc.scalar.activation(psq[:], p_all[:, 0:NCH, 2:2 + D],
                                     AF.Square)
            else:
                nc.vector.tensor_mul(psq[:], p_all[:, 0:NCH, 2:2 + D],
                                     p_all[:, 0:NCH, 2:2 + D])
            pp8 = sb1.tile([128, NCH, 1], F32, tag="pp8")
            nc.vector.reduce_sum(
                pp8[:], psq[:].rearrange("p (c d) -> p c d", c=NCH), axis=AX.X)
            nc.vector.tensor_copy(p_all[:, 0:NCH, 0:1], pp8[:])

        # ============ phase A1: transposes + mm1 + relu/sqrt ===========
        squcols = sb1.tile([128, NC32], F32)
        sqv = squcols[:].rearrange("p (c b2) -> p c b2", b2=BPC)
        d_tiles = []
        for b in range(BPC):
            p_all = p_all_tiles[b]
            # h side from gather chunk 8
            h_aug = sb1.tile([128, 66], F32, tag="h_aug")
            hsq = sb1.tile([128, D], F32, tag="hsq")
            hh = sb1.tile([128, 1], F32, tag="hh")
            nc.vector.tensor_scalar_mul(h_aug[:, 2:2 + D],
                                        p_all[:, NCH, 2:2 + D], -2.0)
            nc.vector.tensor_mul(hsq[:], h_aug[:, 2:2 + D], h_aug[:, 2:2 + D])
            nc.vector.reduce_sum(hh[:], hsq[:], axis=AX.X)
            nc.gpsimd.memset(h_aug[:, 0:1], 1.0)
            nc.vector.tensor_scalar(h_aug[:, 1:2], hh[:], 0.25, EPS,
                                    op0=OP.mult, op1=OP.add)
            ps_h = ps_pT.tile([66, 128], F32, space="PSUM", tag="pT")
            nc.tensor.transpose(ps_h[0:66, 0:128], h_aug[:], ident[:])
            h_side = hsidep.tile([66, 128], MDT1)
            nc.vector.tensor_copy(h_side[:], ps_h[0:66, 0:128])

            # dist_ui^2 via DVE dots: squ = pp - 2 u.p + (uu+eps)
            upt = psqp.tile([128, NCH * D], F32, tag="upt")
            up8 = sb1.tile([128, NCH, 1], F32, tag="up8")
            nc.vector.tensor_mul(
                upt[:].rearrange("p (c d) -> p c d", c=NCH),
                p_all[:, 0:NCH, 2:2 + D],
                u_bc[:, b * D:(b + 1) * D]
                    .rearrange("p (one d) -> p one d", one=1)
                    .to_broadcast([128, NCH, D]))
            nc.vector.reduce_sum(
                up8[:], upt[:].rearrange("p (c d) -> p c d", c=NCH), axis=AX.X)
            nc.vector.scalar_tensor_tensor(
                out=up8[:], in0=up8[:], scalar=-2.0,
                in1=p_all[:, 0:NCH, 0:1], op0=OP.mult, op1=OP.add)
            nc.vector.tensor_scalar_add(sqv[:, :, b:b + 1], up8[:],
                                        uueps_b[:, b:b + 1])

            kt = kchain.tile([128, PP], F32, tag="kt")
            for g in range(2):
                ps_p = ps_pT.tile([67, 512], F32, space="PSUM", tag="pT")
                for cc in range(4):
                    c = g * 4 + cc
                    nc.tensor.transpose(ps_p[0:67, cc * 128:(cc + 1) * 128],
                                        p_all[:, c, 0:67], ident[:])
                p_side = psidep.tile([66, 512], MDT1)
                nc.vector.tensor_copy(p_side[:], ps_p[0:66, :])

                mm = ps_mm.tile([128, 512], F32, space="PSUM", tag="mm")
                nc.tensor.matmul(mm[:], lhsT=h_side[:],
                                 rhs=p_side[:], start=True, stop=True)
                # clamp: relu(sq - eps)  (sqrt adds eps back)
                nc.scalar.activation(kt[:, g * 512:(g + 1) * 512], mm[:],
                                     AF.Relu, bias=-EPS)
            dt_ = kchain.tile([128, PP], F32, tag="dt")
            nc.scalar.activation(dt_[:], kt[:], AF.Sqrt, bias=EPS)
            d_tiles.append(dt_)

        # dist_ui = sqrt(max(squ, eps)) in column space
        squ_r = sb1.tile([128, NC32], F32)
        duij = sb1.tile([128, NC32], F32)
        nc.scalar.activation(squ_r[:], squcols[:], AF.Relu, bias=-EPS)
        nc.scalar.activation(duij[:], squ_r[:], AF.Sqrt, bias=EPS)

        # ============ phase A2: 1/(1+d) ================================
        r_tiles = []
        for b in range(BPC):
            tt_ = kchain.tile([128, PP], F32, tag="tt")
            eng = nc.gpsimd if ADD1_ON_GPS else nc.vector
            eng.tensor_scalar(tt_[:], d_tiles[b][:], inv_smooth, inv_smooth,
                              op0=OP.mult, op1=OP.add)
            rt = kernp.tile([128, PP], F32, tag="rt")
            if ACT_RECIP:
                r0 = kchain.tile([128, PP], F32, tag="r0")
                q = kchain.tile([128, PP], F32, tag="q")
                _raw_activation(nc, r0[:], tt_[:], AF.Reciprocal)
                nc.vector.tensor_mul(q[:], tt_[:], r0[:])
                nc.vector.tensor_scalar(q[:], q[:], -1.0, 2.0,
                                        op0=OP.mult, op1=OP.add)
                nc.vector.tensor_mul(rt[:], q[:], r0[:])
            else:
                nc.vector.reciprocal(rt[:], tt_[:])
            r_tiles.append(rt)

        # ============ phase B: sigmoid + td-reduce =====================
        res4 = [ps_acc.tile([BPC, 512], F32, space="PSUM", tag=f"acc{g}",
                            name=f"res4_{g}")
                for g in range(2)]
        for b in range(BPC):
            kern_t = kernp.tile([128, PP], MDT3, tag="kern")
            nc.scalar.activation(kern_t[:], r_tiles[b][:], AF.Sigmoid,
                                 bias=neg_fs)
            for g in range(2):
                nc.tensor.matmul(res4[g][:],
                                 lhsT=td4m[:, b * BPC:(b + 1) * BPC],
                                 rhs=kern_t[:, g * 512:(g + 1) * 512],
                                 start=(b == 0), stop=(b == BPC - 1),
                                 skip_group_check=True)

        resrows = sb1.tile([BPC, PP], F32)
        for g in range(2):
            nc.vector.tensor_copy(resrows[:, g * 512:(g + 1) * 512],
                                  res4[g][:])
        cols_res = ps_cols.tile([128, NC32], F32, space="PSUM", tag="cols")
        for c in range(NCH):
            nc.tensor.transpose(cols_res[:, c * BPC:(c + 1) * BPC],
                                resrows[:, c * 128:(c + 1) * 128],
                                ident[0:BPC, 0:BPC])

        # ============ finals (column space [128, 32], col = c*4+b) =====
        o1 = sb1.tile([128, NC32], F32)
        o = sb1.tile([128, NC32], F32)
        q2 = sb1.tile([128, NC32], F32)
        m = sb1.tile([128, NC32], F32)
        icols = sb1.tile([128, NC32], F32)
        nc.vector.tensor_sub(o1[:], duij[:], cols_res[:])
        nc.vector.tensor_scalar_max(o[:], o1[:], 0.0)
        nc.vector.tensor_scalar(q2[:], o[:], float(beta), float(alpha),
                                op0=OP.mult, op1=OP.add)
        nc.vector.tensor_mul(m[:], q2[:], o[:])
        mv = m[:].rearrange("p (c b2) -> p c b2", b2=BPC)
        iv = icols[:].rearrange("p (c b2) -> p c b2", b2=BPC)
        for b in range(BPC):
            # + (gamma + user_bias_b) + item_bias   (ib = p_all col 66)
            nc.vector.scalar_tensor_tensor(
                out=iv[:, :, b:b + 1],
                in0=mv[:, :, b:b + 1],
                scalar=ubg_b[:, b:b + 1],
                in1=p_all_tiles[b][:, 0:NCH, 66:67],
                op0=OP.add, op1=OP.add)

        # ---- transpose back to rows [BPC, 1024] and store -------------
        irows = sb1.tile([BPC, PP], F32)
        for g in range(2):
            fin = ps_mm.tile([BPC, 512], F32, space="PSUM", tag="mm")
            for cc in range(4):
                c = g * 4 + cc
                # cols for output chunk c are {c*4+b} -> contiguous slice
                nc.tensor.transpose(fin[0:BPC, cc * 128:(cc + 1) * 128],
                                    icols[:, c * BPC:(c + 1) * BPC], ident[:])
            nc.vector.tensor_copy(irows[:, g * 512:(g + 1) * 512], fin[:])
        nc.sync.dma_start(out=t_out[:], in_=irows[:])

    nc.compile()
    return nc


def _get_nc(scalars):
    key = tuple(float(s) for s in scalars)
    if key not in _cache:
        _cache[key] = _build(key)
    return _cache[key]


def _make_in_maps(inputs):
    user_index = np.asarray(inputs["user_index"]).astype(np.int32)
    pred = np.asarray(inputs["pred_item_indices"]).astype(np.int32)
    hist = np.asarray(inputs["history_item_indices"]).astype(np.int32)
    tdelta = np.asarray(inputs["history_timedeltas"], dtype=np.float32)
    weights = np.asarray(inputs["history_weights"], dtype=np.float32)
    emb_user = np.asarray(inputs["embedding_user"], dtype=np.float32)
    emb_item = np.asarray(inputs["embedding_item"], dtype=np.float32)
    user_lamb = np.asarray(inputs["user_lamb"], dtype=np.float32)
    user_bias = np.asarray(inputs["user_bias"], dtype=np.float32)
    item_bias = np.asarray(inputs["item_bias"], dtype=np.float32)

    aug_item = np.ascontiguousarray(
        np.concatenate([emb_item, item_bias], axis=1), dtype=np.float32)
    user_aug = np.ascontiguousarray(
        np.concatenate([emb_user, user_lamb, user_bias], axis=1),
        dtype=np.float32)

    # pad pred to 1024 cols with index 0 (sliced away on output)
    pred_pad = np.zeros((B, PP), np.int32)
    pred_pad[:, :P_REAL] = pred

    in_maps = []
    for c in range(NCORES):
        sl = slice(c * BPC, (c + 1) * BPC)
        # [128, BPC*9]: per batch row, 8 pred chunk columns + 1 hist column
        pcols = pred_pad[sl].reshape(BPC, NCH, 128).transpose(2, 0, 1)
        idx_ph = np.empty((128, BPC * (NCH + 1)), np.int32)
        for b in range(BPC):
            idx_ph[:, b * (NCH + 1):b * (NCH + 1) + NCH] = pcols[:, b, :]
            idx_ph[:, b * (NCH + 1) + NCH] = hist[c * BPC + b]
        in_maps.append({
            "aug_item": aug_item,
            "user_aug": user_aug,
            "idx_ph": np.ascontiguousarray(idx_ph),
            "idx_user": np.ascontiguousarray(user_index[sl, None]),
            "tdelta": tdelta[sl],
            "tweight": weights[sl],
        })
    return in_maps


def kernel(**inputs) -> np.ndarray:
    scalars = tuple(float(np.asarray(inputs[k])) for k in
                    ("global_lamb", "alpha", "beta", "gamma", "cutoff",
                     "smooth", "force"))
    nc = _get_nc(scalars)
    in_maps = _make_in_maps(inputs)

    res = run_bass_kernel_spmd(
        nc, in_maps, core_ids=list(range(NCORES)),
        trace=bool(int(os.environ.get("K_TRACE", "0"))))
    if res.exec_time_ns is not None:
        kernel.last_exec_time_ns = res.exec_time_ns
    kernel.last_results = res

    out = np.concatenate([res.results[c]["out"][:, :P_REAL]
                          for c in range(NCORES)], axis=0)
    return np.ascontiguousarray(out, dtype=np.float32)


if __name__ == "__main__":
    import reference
    inputs = {k: np.asarray(v) for k, v in reference.setup_inputs().items()}
    expected = np.asarray(reference.reference(**reference.setup_inputs()))
    actual = kernel(**inputs)
    err = np.abs(actual - expected)
    rel = err.max() / np.abs(expected).max()
    print("max abs err:", err.max(), "rel:", rel)
